# revision 5
# baseline (speedup 1.0000x reference)
"""Trainium2 Bass kernel for EnhancedGatedFusion (dense top-2-of-4 MoE + RMSNorm).

Strategy: data-parallel across 8 NeuronCores (one batch of 8192 tokens per
core), weights replicated, no collectives. Per core: 64 tiles of 128 tokens,
processed in 32 pairs.

v2: fp8(e4m3) DoubleRow expert matmuls (4x PE throughput vs bf16 in the
cost model: 2 k-tiles per instruction at 0.5 cycles/row), x transposed via
the DMA xbar engine (bf16) instead of PE+psum-evac, expert inputs cast
bf16->fp8 on gpsimd. Router runs in single bf16 from the same xbT. The
top-2 softmax uses the tanh identity with out_w pre-scaled by 0.5. Combine
uses tensor_scalar (4x DVE mode) + paired adds; final add on gpsimd.
RMSNorm: ACT square+accum, pair-batched Newton rsqrt on DVE.
"""

import numpy as np

import concourse.bass as bass
from concourse import bacc
import concourse.tile as tile
from concourse import mybir
from concourse.bass_utils import run_bass_kernel_spmd
from concourse.masks import make_identity

# Problem shape (hardcoded per harness contract)
B, S, DIM, E, K = 8, 8192, 512, 4, 2
EPS = 1e-6
P = 128
NT = S // P  # token tiles per core (64)
NPAIR = NT // 2  # tile pairs (32)
KT = DIM // P  # contraction k-tiles (4)

F32 = mybir.dt.float32
BF16 = mybir.dt.bfloat16
FP8 = mybir.dt.float8e4
AF = mybir.ActivationFunctionType
OP = mybir.AluOpType
DR = mybir.MatmulPerfMode.DoubleRow

NEG_BIG = -1e30

_cache = {}
TRACE = False
LAST_RESULTS = None


def _build(flags, nt=NT):
    has_rb, has_eb, has_ob, has_nw = flags
    npair = nt // 2
    s_tok = nt * P
    nc = bacc.Bacc()

    x = nc.dram_tensor("x", [s_tok, DIM], F32, kind="ExternalInput")
    router_w = nc.dram_tensor("router_w", [DIM, E], F32, kind="ExternalInput")
    expert_w = nc.dram_tensor("expert_w", [E, DIM, DIM], F32, kind="ExternalInput")
    out_w = nc.dram_tensor("out_w", [DIM, DIM], F32, kind="ExternalInput")
    router_b = nc.dram_tensor("router_b", [E], F32, kind="ExternalInput")
    expert_b = nc.dram_tensor("expert_b", [E, DIM], F32, kind="ExternalInput")
    out_b = nc.dram_tensor("out_b", [DIM], F32, kind="ExternalInput")
    norm_w = nc.dram_tensor("norm_w", [DIM], F32, kind="ExternalInput")
    y_out = nc.dram_tensor("y", [s_tok, DIM], F32, kind="ExternalOutput")

    with tile.TileContext(nc) as tc:
        with (
            tc.tile_pool(name="const", bufs=1) as const,
            tc.tile_pool(name="stage", bufs=4) as stage,
            tc.tile_pool(name="xin", bufs=3) as xin,
            tc.tile_pool(name="xbp", bufs=3) as xbp,
            tc.tile_pool(name="xtp", bufs=6) as xtp,
            tc.tile_pool(name="gp", bufs=2) as gp,
            tc.tile_pool(name="combp", bufs=2) as combp,
            tc.tile_pool(name="yp", bufs=2) as yp,
            tc.tile_pool(name="sm", bufs=3) as sm,
            tc.tile_pool(name="ps_lg", bufs=1, space="PSUM") as ps_lg,
            tc.tile_pool(name="ps_h", bufs=2, space="PSUM") as ps_h,
            tc.tile_pool(name="ps_ct", bufs=1, space="PSUM") as ps_ct,
            tc.tile_pool(name="ps_o", bufs=2, space="PSUM") as ps_o,
        ):
            # ---- constants / weights (one-time) ----
            id_bf16 = const.tile([P, P], BF16)
            make_identity(nc, id_bf16)

            # router weights bf16 [P, KT, E]
            wr_stage = const.tile([P, KT, E], F32)
            nc.sync.dma_start(
                out=wr_stage[:], in_=router_w.rearrange("(k p) e -> p k e", p=P)
            )
            wrouter = const.tile([P, KT, E], BF16)
            nc.vector.tensor_copy(wrouter[:], wr_stage[:])

            if has_rb:
                rb_bc = const.tile([P, E], F32)
                nc.sync.dma_start(out=rb_bc[:], in_=router_b[:].partition_broadcast(P))
            if has_eb:
                eb_bc = const.tile([P, E, DIM], F32)
                nc.sync.dma_start(out=eb_bc[:], in_=expert_b[:, :].partition_broadcast(P))
            if has_ob:
                ob_bc = const.tile([P, DIM], F32)
                nc.sync.dma_start(out=ob_bc[:], in_=out_b[:].partition_broadcast(P))
            if has_nw:
                nw_bc = const.tile([P, DIM], F32)
                nc.sync.dma_start(out=nw_bc[:], in_=norm_w[:].partition_broadcast(P))

            # persistent logits scratch with -inf padding at [4:8] and [12:16]
            lg8 = const.tile([P, 16], F32)
            nc.vector.memset(lg8[:, 4:8], NEG_BIG)
            nc.vector.memset(lg8[:, 12:16], NEG_BIG)

            # ---- stage A: per-pair input load / cast / transpose ----
            def stage_a(p):
                x_pair = xin.tile([P, 2, DIM], F32, tag="x")
                nc.sync.dma_start(
                    out=x_pair[:],
                    in_=x[p * 2 * P : (p + 1) * 2 * P, :].rearrange(
                        "(a q) d -> q a d", a=2
                    ),
                )
                xb_pair = xbp.tile([P, 2 * DIM], BF16, tag="xb")
                nc.gpsimd.tensor_copy(
                    xb_pair[:], x_pair[:].rearrange("q a d -> q (a d)")
                )
                xbt = xtp.tile([P, 8, P], BF16, tag="xbt")
                nc.sync.dma_start_transpose(xbt[:], xb_pair[:])
                x8t = xtp.tile([P, 8, P], FP8, tag="x8t")
                nc.gpsimd.tensor_copy(x8t[:], xbt[:])
                return x_pair, xbt, x8t

            staged = {pp: stage_a(pp) for pp in range(min(2, npair))}

            # expert weights fp8 [P, KT, DIM] per expert
            we_sb = []
            for e in range(E):
                we_e = const.tile([P, KT, DIM], FP8, tag=f"we{e}")
                for kt in range(KT):
                    st = stage.tile([P, DIM], F32, tag="wstage")
                    nc.sync.dma_start(
                        out=st[:], in_=expert_w[e, kt * P : (kt + 1) * P, :]
                    )
                    nc.vector.tensor_copy(we_e[:, kt, :], st[:])
                we_sb.append(we_e)

            # out_w bf16, pre-scaled by 0.5 (softmax-via-tanh factor)
            ow_sb = const.tile([P, KT, DIM], BF16)
            for kt in range(KT):
                st = stage.tile([P, DIM], F32, tag="wstage")
                nc.sync.dma_start(out=st[:], in_=out_w[kt * P : (kt + 1) * P, :])
                nc.vector.tensor_scalar_mul(ow_sb[:, kt, :], st[:], 0.5)

            # ---- main loop over pairs ----
            for pp in range(npair):
                if pp + 2 < npair and pp + 2 not in staged:
                    staged[pp + 2] = stage_a(pp + 2)
                elif pp + 1 < npair and pp + 1 not in staged:
                    staged[pp + 1] = stage_a(pp + 1)
                x_pair, xbt, x8t = staged.pop(pp)

                # router logits for both tiles into one psum bank
                plg = ps_lg.tile([P, 8], F32, tag="plg")
                for i in range(2):
                    for kt in range(KT):
                        nc.tensor.matmul(
                            plg[:, 4 * i : 4 * i + 4],
                            xbt[:, 4 * i + kt, :],
                            wrouter[:, kt, :],
                            start=(kt == 0),
                            stop=(kt == KT - 1),
                        )
                # evac both tiles' logits into padded sbuf scratch
                nc.vector.tensor_copy(
                    lg8[:].rearrange("q (a d) -> q a d", a=2)[:, :, 0:4],
                    plg[:].rearrange("q (a d) -> q a d", a=2),
                )
                if has_rb:
                    for i in range(2):
                        nc.vector.tensor_add(
                            lg8[:, 8 * i : 8 * i + 4],
                            lg8[:, 8 * i : 8 * i + 4],
                            rb_bc[:],
                        )

                mx = sm.tile([P, 16], F32, tag="mx")
                u = sm.tile([P, 2, E], F32, tag="u")
                bs = sm.tile([P, 2], F32, tag="bs")
                tnh = sm.tile([P, 2, E], F32, tag="tnh")
                mask = sm.tile([P, 2, E], F32, tag="mask")
                y_pair = yp.tile([P, 2, DIM], F32, tag="ypair")
                yo_pair = yp.tile([P, 2, DIM], F32, tag="yopair")
                ssq = sm.tile([P, 2], F32, tag="ssq")

                for i in range(2):
                    nc.vector.max(out=mx[:, 8 * i : 8 * i + 8], in_=lg8[:, 8 * i : 8 * i + 8])
                    # bs = -(mx0 + mx1)/2
                    nc.vector.tensor_scalar(
                        bs[:, i : i + 1],
                        mx[:, 8 * i : 8 * i + 1],
                        mx[:, 8 * i + 1 : 8 * i + 2],
                        -0.5,
                        op0=OP.add,
                        op1=OP.mult,
                    )
                    nc.scalar.activation(
                        tnh[:, i, :], lg8[:, 8 * i : 8 * i + 4], AF.Tanh,
                        bias=bs[:, i : i + 1],
                    )
                    nc.vector.tensor_scalar(
                        mask[:, i, :], lg8[:, 8 * i : 8 * i + 4],
                        mx[:, 8 * i + 1 : 8 * i + 2], None, op0=OP.is_ge,
                    )
                    # u = (tnh + 1) * mask   (0.5 folded into out_w)
                    nc.vector.scalar_tensor_tensor(
                        out=u[:, i, :], in0=tnh[:, i, :], scalar=1.0,
                        in1=mask[:, i, :], op0=OP.add, op1=OP.mult,
                    )

                for i in range(2):
                    # experts: h_e = X @ W_e, fp8 DoubleRow, 2 psum tiles
                    ph01 = ps_h.tile([P, 2, DIM], F32, tag="ph")
                    ph23 = ps_h.tile([P, 2, DIM], F32, tag="ph")
                    phs = {0: ph01[:, 0, :], 1: ph01[:, 1, :],
                           2: ph23[:, 0, :], 3: ph23[:, 1, :]}
                    for e in range(E):
                        for g in range(2):
                            nc.tensor.matmul(
                                phs[e],
                                x8t[:, 4 * i + 2 * g : 4 * i + 2 * g + 2, :],
                                we_sb[e][:, 2 * g : 2 * g + 2, :],
                                start=(g == 0),
                                stop=(g == 1),
                                perf_mode=DR,
                            )
                    # silu (batched 2 experts per ACT op)
                    gt = gp.tile([P, E, DIM], BF16, tag="g")
                    if has_eb:
                        hb01 = gp.tile([P, 2, DIM], F32, tag="hb01")
                        nc.vector.tensor_add(hb01[:], ph01[:], eb_bc[:, 0:2, :])
                        nc.scalar.activation(
                            gt[:, 0:2, :].rearrange("q a d -> q (a d)"),
                            hb01[:].rearrange("q a d -> q (a d)"), AF.Silu)
                        hb23 = gp.tile([P, 2, DIM], F32, tag="hb23")
                        nc.vector.tensor_add(hb23[:], ph23[:], eb_bc[:, 2:4, :])
                        nc.scalar.activation(
                            gt[:, 2:4, :].rearrange("q a d -> q (a d)"),
                            hb23[:].rearrange("q a d -> q (a d)"), AF.Silu)
                    else:
                        nc.scalar.activation(
                            gt[:, 0:2, :].rearrange("q a d -> q (a d)"),
                            ph01[:].rearrange("q a d -> q (a d)"), AF.Silu)
                        nc.scalar.activation(
                            gt[:, 2:4, :].rearrange("q a d -> q (a d)"),
                            ph23[:].rearrange("q a d -> q (a d)"), AF.Silu)

                    # weighted combine: gw_e = u_e * g_e (DVE 4x), pair-add,
                    # final add on gpsimd
                    gw = gp.tile([P, E, DIM], BF16, tag="gw")
                    for e in range(E):
                        nc.vector.tensor_scalar_mul(
                            gw[:, e, :], gt[:, e, :], u[:, i, e : e + 1]
                        )
                    c2 = combp.tile([P, 2, DIM], BF16, tag="c2")
                    nc.vector.tensor_tensor(
                        out=c2[:], in0=gw[:, 0:2, :], in1=gw[:, 2:4, :], op=OP.add
                    )
                    comb = combp.tile([P, DIM], BF16, tag="comb")
                    nc.gpsimd.tensor_tensor(
                        out=comb[:], in0=c2[:, 0, :], in1=c2[:, 1, :], op=OP.add
                    )

                    # transpose comb (bf16) on PE
                    pct = ps_ct.tile([P, KT, P], BF16, tag="pct")
                    for j in range(KT):
                        nc.tensor.transpose(
                            pct[:, j, :],
                            comb[:, j * P : (j + 1) * P],
                            id_bf16[:],
                        )
                    combT = combp.tile([P, KT, P], BF16, tag="combT")
                    nc.vector.tensor_copy(combT[:], pct[:])

                    # out projection: out = comb @ (0.5*out_w)
                    po = ps_o.tile([P, DIM], F32, tag="po")
                    for kt in range(KT):
                        nc.tensor.matmul(
                            po[:],
                            combT[:, kt, :],
                            ow_sb[:, kt, :],
                            start=(kt == 0),
                            stop=(kt == KT - 1),
                        )

                    # residual
                    nc.vector.tensor_tensor(
                        out=y_pair[:, i, :], in0=po[:], in1=x_pair[:, i, :], op=OP.add
                    )
                    if has_ob:
                        nc.vector.tensor_add(y_pair[:, i, :], y_pair[:, i, :], ob_bc[:])
                    scr = yp.tile([P, DIM], BF16, tag="scr")
                    nc.scalar.activation(
                        scr[:], y_pair[:, i, :], AF.Square, accum_out=ssq[:, i : i + 1]
                    )

                # rsqrt via linear seed + 2 Newton steps, batched for the pair
                nr = sm.tile([P, 8], F32, tag="nr")
                m_ = nr[:, 0:2]
                r_ = nr[:, 2:4]
                t_ = nr[:, 4:6]
                f_ = nr[:, 6:8]
                nc.vector.tensor_scalar(
                    m_, ssq[:], 1.0 / DIM, EPS, op0=OP.mult, op1=OP.add
                )
                nc.vector.tensor_scalar(
                    r_, ssq[:], -0.5 / DIM, 1.5 - 0.5 * EPS, op0=OP.mult, op1=OP.add
                )
                for it in range(2):
                    nc.vector.tensor_mul(t_, r_, r_)
                    nc.vector.tensor_mul(t_, t_, m_)
                    nc.vector.tensor_scalar(
                        f_, t_, -0.5, 1.5, op0=OP.mult, op1=OP.add
                    )
                    nc.vector.tensor_mul(r_, r_, f_)

                for i in range(2):
                    if has_nw:
                        nc.vector.tensor_mul(
                            yo_pair[:, i, :], y_pair[:, i, :], nw_bc[:]
                        )
                        nc.vector.tensor_scalar_mul(
                            yo_pair[:, i, :], yo_pair[:, i, :], r_[:, i : i + 1]
                        )
                    else:
                        nc.vector.tensor_scalar_mul(
                            yo_pair[:, i, :], y_pair[:, i, :], r_[:, i : i + 1]
                        )

                nc.sync.dma_start(
                    out=y_out[pp * 2 * P : (pp + 1) * 2 * P, :].rearrange(
                        "(a q) d -> q a d", a=2
                    ),
                    in_=yo_pair[:],
                )

    nc.compile()
    return nc


def _get_nc(flags):
    if flags not in _cache:
        _cache[flags] = _build(flags)
    return _cache[flags]


def kernel(x, router_w, router_b, expert_w, expert_b, out_w, out_b, norm_w):
    x = np.ascontiguousarray(np.asarray(x, dtype=np.float32))
    router_w = np.ascontiguousarray(np.asarray(router_w, dtype=np.float32))
    router_b = np.ascontiguousarray(np.asarray(router_b, dtype=np.float32))
    expert_w = np.ascontiguousarray(np.asarray(expert_w, dtype=np.float32))
    expert_b = np.ascontiguousarray(np.asarray(expert_b, dtype=np.float32))
    out_w = np.ascontiguousarray(np.asarray(out_w, dtype=np.float32))
    out_b = np.ascontiguousarray(np.asarray(out_b, dtype=np.float32))
    norm_w = np.ascontiguousarray(np.asarray(norm_w, dtype=np.float32))

    flags = (
        bool(np.any(router_b != 0.0)),
        bool(np.any(expert_b != 0.0)),
        bool(np.any(out_b != 0.0)),
        bool(np.any(norm_w != 1.0)),
    )
    nc = _get_nc(flags)

    shared = {
        "router_w": router_w,
        "expert_w": expert_w,
        "out_w": out_w,
        "router_b": router_b,
        "expert_b": expert_b,
        "out_b": out_b,
        "norm_w": norm_w,
    }
    runner = _get_runner(flags)
    return runner(x, shared)


_runners = {}


def _get_runner(flags):
    """Persistent jitted SPMD runner (avoids re-lowering on every call)."""
    if flags in _runners:
        return _runners[flags]
    import jax
    from jax.sharding import Mesh, PartitionSpec, NamedSharding
    from jax.experimental.shard_map import shard_map
    from concourse.bass2jax import (
        _bass_exec_p,
        install_neuronx_cc_hook,
        partition_id_tensor,
    )

    nc = _get_nc(flags)
    install_neuronx_cc_hook()
    in_names, out_names, out_avals, zero_shapes = [], [], [], []
    for alloc in nc.m.functions[0].allocations:
        if not isinstance(alloc, mybir.MemoryLocationSet):
            continue
        name = alloc.memorylocations[0].name
        if alloc.kind == "ExternalInput":
            if nc.partition_id_tensor is None or name != nc.partition_id_tensor.name:
                in_names.append(name)
        elif alloc.kind == "ExternalOutput":
            out_names.append(name)
            shape = tuple(alloc.tensor_shape)
            dtype = mybir.dt.np(alloc.dtype)
            out_avals.append(jax.core.ShapedArray(shape, dtype))
            zero_shapes.append((shape, dtype))
    n_params = len(in_names)
    has_pid = nc.partition_id_tensor is not None
    all_in_names = in_names + out_names
    if has_pid:
        all_in_names = all_in_names + [nc.partition_id_tensor.name]

    def _body(*args):
        operands = list(args)
        if has_pid:
            operands.append(partition_id_tensor())
        outs = _bass_exec_p.bind(
            *operands,
            out_avals=tuple(out_avals),
            in_names=tuple(all_in_names),
            out_names=tuple(out_names),
            lowering_input_output_aliases=(),
            sim_require_finite=True,
            sim_require_nnan=True,
            nc=nc,
        )
        return tuple(outs)

    devices = jax.devices()[:B]
    mesh = Mesh(np.asarray(devices), ("core",))
    n_outs = len(out_names)
    sharded = jax.jit(
        shard_map(
            _body,
            mesh=mesh,
            in_specs=(PartitionSpec("core"),) * (n_params + n_outs),
            out_specs=(PartitionSpec("core"),) * n_outs,
            check_rep=False,
        ),
        donate_argnums=tuple(range(n_params, n_params + n_outs)),
        keep_unused=True,
    )
    sh = NamedSharding(mesh, PartitionSpec("core"))
    yi = out_names.index("y")

    def run(x_full, shared):
        concat = []
        for name in in_names:
            if name == "x":
                concat.append(x_full.reshape(B * S, DIM))
            else:
                concat.append(np.concatenate([shared[name]] * B, axis=0))
        dev_in = [jax.device_put(a, sh) for a in concat]
        zeros = [
            jax.device_put(np.zeros((B * z[0][0], *z[0][1:]), z[1]), sh)
            for z in zero_shapes
        ]
        outs = sharded(*dev_in, *zeros)
        y = np.asarray(outs[yi]).reshape(B, S, DIM)
        return y

    _runners[flags] = run
    return run


if __name__ == "__main__":
    rng = np.random.default_rng(0)
    inp = {
        "x": rng.standard_normal((B, S, DIM), dtype=np.float32),
        "router_w": (rng.standard_normal((DIM, E)) * 0.02).astype(np.float32),
        "router_b": np.zeros(E, np.float32),
        "expert_w": (rng.standard_normal((E, DIM, DIM)) * 0.02).astype(np.float32),
        "expert_b": np.zeros((E, DIM), np.float32),
        "out_w": (rng.standard_normal((DIM, DIM)) * 0.02).astype(np.float32),
        "out_b": np.zeros(DIM, np.float32),
        "norm_w": np.ones(DIM, np.float32),
    }
    y = kernel(**inp)
    print("kernel ran, y shape", y.shape, "finite:", np.isfinite(y).all())


# revision 8
# speedup vs baseline: 1.0125x; 1.0125x over previous
"""Trainium2 Bass kernel for EnhancedGatedFusion (dense top-2-of-4 MoE + RMSNorm).

Strategy: data-parallel across 8 NeuronCores (one batch of 8192 tokens per
core), weights replicated, no collectives. Per core: 64 tiles of 128 tokens,
processed in 32 pairs.

v2: fp8(e4m3) DoubleRow expert matmuls (4x PE throughput vs bf16 in the
cost model: 2 k-tiles per instruction at 0.5 cycles/row), x transposed via
the DMA xbar engine (bf16) instead of PE+psum-evac, expert inputs cast
bf16->fp8 on gpsimd. Router runs in single bf16 from the same xbT. The
top-2 softmax uses the tanh identity with out_w pre-scaled by 0.5. Combine
uses tensor_scalar (4x DVE mode) + paired adds; final add on gpsimd.
RMSNorm: ACT square+accum, pair-batched Newton rsqrt on DVE.
"""

import numpy as np

import concourse.bass as bass
from concourse import bacc
import concourse.tile as tile
from concourse import mybir
from concourse.bass_utils import run_bass_kernel_spmd
from concourse.masks import make_identity

# Problem shape (hardcoded per harness contract)
B, S, DIM, E, K = 8, 8192, 512, 4, 2
EPS = 1e-6
P = 128
NT = S // P  # token tiles per core (64)
NPAIR = NT // 2  # tile pairs (32)
KT = DIM // P  # contraction k-tiles (4)

F32 = mybir.dt.float32
BF16 = mybir.dt.bfloat16
FP8 = mybir.dt.float8e4
AF = mybir.ActivationFunctionType
OP = mybir.AluOpType
DR = mybir.MatmulPerfMode.DoubleRow

NEG_BIG = -1e30

_cache = {}
TRACE = False
LAST_RESULTS = None


def _build(flags, nt=NT):
    has_rb, has_eb, has_ob, has_nw = flags
    npair = nt // 2
    s_tok = nt * P
    nc = bacc.Bacc()

    x = nc.dram_tensor("x", [s_tok, DIM], F32, kind="ExternalInput")
    router_w = nc.dram_tensor("router_w", [DIM, E], F32, kind="ExternalInput")
    expert_w = nc.dram_tensor("expert_w", [E, DIM, DIM], F32, kind="ExternalInput")
    out_w = nc.dram_tensor("out_w", [DIM, DIM], F32, kind="ExternalInput")
    router_b = nc.dram_tensor("router_b", [E], F32, kind="ExternalInput")
    expert_b = nc.dram_tensor("expert_b", [E, DIM], F32, kind="ExternalInput")
    out_b = nc.dram_tensor("out_b", [DIM], F32, kind="ExternalInput")
    norm_w = nc.dram_tensor("norm_w", [DIM], F32, kind="ExternalInput")
    y_out = nc.dram_tensor("y", [s_tok, DIM], F32, kind="ExternalOutput")

    with tile.TileContext(nc) as tc:
        with (
            tc.tile_pool(name="const", bufs=1) as const,
            tc.tile_pool(name="stage", bufs=4) as stage,
            tc.tile_pool(name="xin", bufs=4) as xin,
            tc.tile_pool(name="xbp", bufs=3) as xbp,
            tc.tile_pool(name="xtp", bufs=6) as xtp,
            tc.tile_pool(name="gp", bufs=2) as gp,
            tc.tile_pool(name="combp", bufs=2) as combp,
            tc.tile_pool(name="yp", bufs=2) as yp,
            tc.tile_pool(name="sm", bufs=3) as sm,
            tc.tile_pool(name="ps_lg", bufs=1, space="PSUM") as ps_lg,
            tc.tile_pool(name="ps_h", bufs=2, space="PSUM") as ps_h,
            tc.tile_pool(name="ps_ct", bufs=1, space="PSUM") as ps_ct,
            tc.tile_pool(name="ps_o", bufs=2, space="PSUM") as ps_o,
        ):
            # ---- constants / weights (one-time) ----
            id_bf16 = const.tile([P, P], BF16)
            make_identity(nc, id_bf16)

            # router weights bf16 [P, KT, E]
            wr_stage = const.tile([P, KT, E], F32)
            nc.sync.dma_start(
                out=wr_stage[:], in_=router_w.rearrange("(k p) e -> p k e", p=P)
            )
            wrouter = const.tile([P, KT, E], BF16)
            nc.vector.tensor_copy(wrouter[:], wr_stage[:])

            if has_rb:
                rb_bc = const.tile([P, E], F32)
                nc.sync.dma_start(out=rb_bc[:], in_=router_b[:].partition_broadcast(P))
            if has_eb:
                eb_bc = const.tile([P, E, DIM], F32)
                nc.sync.dma_start(out=eb_bc[:], in_=expert_b[:, :].partition_broadcast(P))
            if has_ob:
                ob_bc = const.tile([P, DIM], F32)
                nc.sync.dma_start(out=ob_bc[:], in_=out_b[:].partition_broadcast(P))
            if has_nw:
                nw_bc = const.tile([P, DIM], F32)
                nc.sync.dma_start(out=nw_bc[:], in_=norm_w[:].partition_broadcast(P))

            # persistent logits scratch (ping-pong pair) with -inf padding
            # at [4:8] and [12:16]
            lg8_pp = []
            for _ in range(2):
                lg8 = const.tile([P, 16], F32, tag=f"lg8_{_}")
                nc.vector.memset(lg8[:, 4:8], NEG_BIG)
                nc.vector.memset(lg8[:, 12:16], NEG_BIG)
                lg8_pp.append(lg8)

            # ---- stage A: per-pair input load / cast / transpose ----
            def stage_a(p):
                x_pair = xin.tile([P, 2, DIM], F32, tag="x")
                nc.sync.dma_start(
                    out=x_pair[:],
                    in_=x[p * 2 * P : (p + 1) * 2 * P, :].rearrange(
                        "(a q) d -> q a d", a=2
                    ),
                )
                xb_pair = xbp.tile([P, 2 * DIM], BF16, tag="xb")
                nc.gpsimd.tensor_copy(
                    xb_pair[:], x_pair[:].rearrange("q a d -> q (a d)")
                )
                xbt = xtp.tile([P, 8, P], BF16, tag="xbt")
                nc.sync.dma_start_transpose(xbt[:], xb_pair[:])
                x8t = xtp.tile([P, 8, P], FP8, tag="x8t")
                nc.gpsimd.tensor_copy(x8t[:], xbt[:])
                return x_pair, xbt, x8t

            staged = {pp: stage_a(pp) for pp in range(min(2, npair))}

            # expert weights fp8 [P, KT, DIM] per expert
            we_sb = []
            for e in range(E):
                we_e = const.tile([P, KT, DIM], FP8, tag=f"we{e}")
                for kt in range(KT):
                    st = stage.tile([P, DIM], F32, tag="wstage")
                    nc.sync.dma_start(
                        out=st[:], in_=expert_w[e, kt * P : (kt + 1) * P, :]
                    )
                    nc.vector.tensor_copy(we_e[:, kt, :], st[:])
                we_sb.append(we_e)

            # out_w bf16, pre-scaled by 0.5 (softmax-via-tanh factor)
            ow_sb = const.tile([P, KT, DIM], BF16)
            for kt in range(KT):
                st = stage.tile([P, DIM], F32, tag="wstage")
                nc.sync.dma_start(out=st[:], in_=out_w[kt * P : (kt + 1) * P, :])
                nc.vector.tensor_scalar_mul(ow_sb[:, kt, :], st[:], 0.5)

            # ---- tail stage: residual + rmsnorm + store for pair pp ----
            def tail(state, pp):
                x_pair, po01 = state
                y_pair = yp.tile([P, 2, DIM], F32, tag="ypair")
                yo_pair = yp.tile([P, 2, DIM], F32, tag="yopair")
                ssq = sm.tile([P, 2], F32, tag="ssq")
                for i in range(2):
                    nc.vector.tensor_tensor(
                        out=y_pair[:, i, :], in0=po01[i][:], in1=x_pair[:, i, :],
                        op=OP.add,
                    )
                    if has_ob:
                        nc.vector.tensor_add(y_pair[:, i, :], y_pair[:, i, :], ob_bc[:])
                    scr = yp.tile([P, DIM], BF16, tag="scr")
                    nc.scalar.activation(
                        scr[:], y_pair[:, i, :], AF.Square, accum_out=ssq[:, i : i + 1]
                    )
                # rsqrt via linear seed + 2 Newton steps, batched for the pair
                nr = sm.tile([P, 8], F32, tag="nr")
                m_ = nr[:, 0:2]
                r_ = nr[:, 2:4]
                t_ = nr[:, 4:6]
                f_ = nr[:, 6:8]
                nc.vector.tensor_scalar(
                    m_, ssq[:], 1.0 / DIM, EPS, op0=OP.mult, op1=OP.add
                )
                nc.vector.tensor_scalar(
                    r_, ssq[:], -0.5 / DIM, 1.5 - 0.5 * EPS, op0=OP.mult, op1=OP.add
                )
                for it in range(2):
                    nc.vector.tensor_mul(t_, r_, r_)
                    nc.vector.tensor_mul(t_, t_, m_)
                    nc.vector.tensor_scalar(
                        f_, t_, -0.5, 1.5, op0=OP.mult, op1=OP.add
                    )
                    nc.vector.tensor_mul(r_, r_, f_)
                for i in range(2):
                    if has_nw:
                        nc.vector.tensor_mul(
                            yo_pair[:, i, :], y_pair[:, i, :], nw_bc[:]
                        )
                        nc.vector.tensor_scalar_mul(
                            yo_pair[:, i, :], yo_pair[:, i, :], r_[:, i : i + 1]
                        )
                    else:
                        nc.vector.tensor_scalar_mul(
                            yo_pair[:, i, :], y_pair[:, i, :], r_[:, i : i + 1]
                        )
                nc.sync.dma_start(
                    out=y_out[pp * 2 * P : (pp + 1) * 2 * P, :].rearrange(
                        "(a q) d -> q a d", a=2
                    ),
                    in_=yo_pair[:],
                )

            # ---- front stage: router + experts + combine + out-proj ----
            def front(pp):
                x_pair, xbt, x8t = staged.pop(pp)
                lg8 = lg8_pp[pp % 2]

                # router logits for both tiles into one psum bank
                plg = ps_lg.tile([P, 8], F32, tag="plg")
                for i in range(2):
                    for kt in range(KT):
                        nc.tensor.matmul(
                            plg[:, 4 * i : 4 * i + 4],
                            xbt[:, 4 * i + kt, :],
                            wrouter[:, kt, :],
                            start=(kt == 0),
                            stop=(kt == KT - 1),
                        )
                # evac both tiles' logits into padded sbuf scratch
                nc.vector.tensor_copy(
                    lg8[:].rearrange("q (a d) -> q a d", a=2)[:, :, 0:4],
                    plg[:].rearrange("q (a d) -> q a d", a=2),
                )
                if has_rb:
                    for i in range(2):
                        nc.vector.tensor_add(
                            lg8[:, 8 * i : 8 * i + 4],
                            lg8[:, 8 * i : 8 * i + 4],
                            rb_bc[:],
                        )

                mx = sm.tile([P, 16], F32, tag="mx")
                u = sm.tile([P, 2, E], F32, tag="u")
                bs = sm.tile([P, 2], F32, tag="bs")
                tnh = sm.tile([P, 2, E], F32, tag="tnh")
                mask = sm.tile([P, 2, E], F32, tag="mask")

                for i in range(2):
                    nc.vector.max(out=mx[:, 8 * i : 8 * i + 8], in_=lg8[:, 8 * i : 8 * i + 8])
                    # bs = -(mx0 + mx1)/2
                    nc.vector.tensor_scalar(
                        bs[:, i : i + 1],
                        mx[:, 8 * i : 8 * i + 1],
                        mx[:, 8 * i + 1 : 8 * i + 2],
                        -0.5,
                        op0=OP.add,
                        op1=OP.mult,
                    )
                    nc.scalar.activation(
                        tnh[:, i, :], lg8[:, 8 * i : 8 * i + 4], AF.Tanh,
                        bias=bs[:, i : i + 1],
                    )
                    nc.vector.tensor_scalar(
                        mask[:, i, :], lg8[:, 8 * i : 8 * i + 4],
                        mx[:, 8 * i + 1 : 8 * i + 2], None, op0=OP.is_ge,
                    )
                    # u = (tnh + 1) * mask   (0.5 folded into out_w)
                    nc.vector.scalar_tensor_tensor(
                        out=u[:, i, :], in0=tnh[:, i, :], scalar=1.0,
                        in1=mask[:, i, :], op0=OP.add, op1=OP.mult,
                    )

                po01 = []
                for i in range(2):
                    # experts: h_e = X @ W_e, fp8 DoubleRow, 2 psum tiles
                    ph01 = ps_h.tile([P, 2, DIM], F32, tag="ph")
                    ph23 = ps_h.tile([P, 2, DIM], F32, tag="ph")
                    phs = {0: ph01[:, 0, :], 1: ph01[:, 1, :],
                           2: ph23[:, 0, :], 3: ph23[:, 1, :]}
                    for e in range(E):
                        for g in range(2):
                            nc.tensor.matmul(
                                phs[e],
                                x8t[:, 4 * i + 2 * g : 4 * i + 2 * g + 2, :],
                                we_sb[e][:, 2 * g : 2 * g + 2, :],
                                start=(g == 0),
                                stop=(g == 1),
                                perf_mode=DR,
                            )
                    # silu (batched 2 experts per ACT op)
                    gt = gp.tile([P, E, DIM], BF16, tag="g")
                    if has_eb:
                        hb01 = gp.tile([P, 2, DIM], F32, tag="hb01")
                        nc.vector.tensor_add(hb01[:], ph01[:], eb_bc[:, 0:2, :])
                        nc.scalar.activation(
                            gt[:, 0:2, :].rearrange("q a d -> q (a d)"),
                            hb01[:].rearrange("q a d -> q (a d)"), AF.Silu)
                        hb23 = gp.tile([P, 2, DIM], F32, tag="hb23")
                        nc.vector.tensor_add(hb23[:], ph23[:], eb_bc[:, 2:4, :])
                        nc.scalar.activation(
                            gt[:, 2:4, :].rearrange("q a d -> q (a d)"),
                            hb23[:].rearrange("q a d -> q (a d)"), AF.Silu)
                    else:
                        nc.scalar.activation(
                            gt[:, 0:2, :].rearrange("q a d -> q (a d)"),
                            ph01[:].rearrange("q a d -> q (a d)"), AF.Silu)
                        nc.scalar.activation(
                            gt[:, 2:4, :].rearrange("q a d -> q (a d)"),
                            ph23[:].rearrange("q a d -> q (a d)"), AF.Silu)

                    # weighted combine: gw_e = u_e * g_e (DVE 4x), pair-add,
                    # final add on gpsimd
                    gw = gp.tile([P, E, DIM], BF16, tag="gw")
                    for e in range(E):
                        nc.vector.tensor_scalar_mul(
                            gw[:, e, :], gt[:, e, :], u[:, i, e : e + 1]
                        )
                    c2 = combp.tile([P, 2, DIM], BF16, tag="c2")
                    nc.vector.tensor_tensor(
                        out=c2[:], in0=gw[:, 0:2, :], in1=gw[:, 2:4, :], op=OP.add
                    )
                    comb = combp.tile([P, DIM], BF16, tag="comb")
                    nc.gpsimd.tensor_tensor(
                        out=comb[:], in0=c2[:, 0, :], in1=c2[:, 1, :], op=OP.add
                    )

                    # transpose comb (bf16) on PE
                    pct = ps_ct.tile([P, KT, P], BF16, tag="pct")
                    for j in range(KT):
                        nc.tensor.transpose(
                            pct[:, j, :],
                            comb[:, j * P : (j + 1) * P],
                            id_bf16[:],
                        )
                    combT = combp.tile([P, KT, P], BF16, tag="combT")
                    nc.vector.tensor_copy(combT[:], pct[:])

                    # out projection: out = comb @ (0.5*out_w)
                    po = ps_o.tile([P, DIM], F32, tag="po")
                    for kt in range(KT):
                        nc.tensor.matmul(
                            po[:],
                            combT[:, kt, :],
                            ow_sb[:, kt, :],
                            start=(kt == 0),
                            stop=(kt == KT - 1),
                        )
                    po01.append(po)
                return (x_pair, po01)

            # ---- main loop: front(pp) and tail(pp-1) interleaved ----
            pending = {}
            for pp in range(npair + 1):
                if pp + 2 < npair and pp + 2 not in staged:
                    staged[pp + 2] = stage_a(pp + 2)
                if pp - 1 in pending:
                    tail(pending.pop(pp - 1), pp - 1)
                if pp < npair:
                    pending[pp] = front(pp)

    nc.compile()
    return nc


def _get_nc(flags):
    if flags not in _cache:
        _cache[flags] = _build(flags)
    return _cache[flags]


def kernel(x, router_w, router_b, expert_w, expert_b, out_w, out_b, norm_w):
    x = np.ascontiguousarray(np.asarray(x, dtype=np.float32))
    router_w = np.ascontiguousarray(np.asarray(router_w, dtype=np.float32))
    router_b = np.ascontiguousarray(np.asarray(router_b, dtype=np.float32))
    expert_w = np.ascontiguousarray(np.asarray(expert_w, dtype=np.float32))
    expert_b = np.ascontiguousarray(np.asarray(expert_b, dtype=np.float32))
    out_w = np.ascontiguousarray(np.asarray(out_w, dtype=np.float32))
    out_b = np.ascontiguousarray(np.asarray(out_b, dtype=np.float32))
    norm_w = np.ascontiguousarray(np.asarray(norm_w, dtype=np.float32))

    flags = (
        bool(np.any(router_b != 0.0)),
        bool(np.any(expert_b != 0.0)),
        bool(np.any(out_b != 0.0)),
        bool(np.any(norm_w != 1.0)),
    )
    nc = _get_nc(flags)

    shared = {
        "router_w": router_w,
        "expert_w": expert_w,
        "out_w": out_w,
        "router_b": router_b,
        "expert_b": expert_b,
        "out_b": out_b,
        "norm_w": norm_w,
    }
    runner = _get_runner(flags)
    return runner(x, shared)


_runners = {}


def _get_runner(flags):
    """Persistent jitted SPMD runner (avoids re-lowering on every call)."""
    if flags in _runners:
        return _runners[flags]
    import jax
    from jax.sharding import Mesh, PartitionSpec, NamedSharding
    from jax.experimental.shard_map import shard_map
    from concourse.bass2jax import (
        _bass_exec_p,
        install_neuronx_cc_hook,
        partition_id_tensor,
    )

    nc = _get_nc(flags)
    install_neuronx_cc_hook()
    in_names, out_names, out_avals, zero_shapes = [], [], [], []
    for alloc in nc.m.functions[0].allocations:
        if not isinstance(alloc, mybir.MemoryLocationSet):
            continue
        name = alloc.memorylocations[0].name
        if alloc.kind == "ExternalInput":
            if nc.partition_id_tensor is None or name != nc.partition_id_tensor.name:
                in_names.append(name)
        elif alloc.kind == "ExternalOutput":
            out_names.append(name)
            shape = tuple(alloc.tensor_shape)
            dtype = mybir.dt.np(alloc.dtype)
            out_avals.append(jax.core.ShapedArray(shape, dtype))
            zero_shapes.append((shape, dtype))
    n_params = len(in_names)
    has_pid = nc.partition_id_tensor is not None
    all_in_names = in_names + out_names
    if has_pid:
        all_in_names = all_in_names + [nc.partition_id_tensor.name]

    def _body(*args):
        operands = list(args)
        if has_pid:
            operands.append(partition_id_tensor())
        outs = _bass_exec_p.bind(
            *operands,
            out_avals=tuple(out_avals),
            in_names=tuple(all_in_names),
            out_names=tuple(out_names),
            lowering_input_output_aliases=(),
            sim_require_finite=True,
            sim_require_nnan=True,
            nc=nc,
        )
        return tuple(outs)

    devices = jax.devices()[:B]
    mesh = Mesh(np.asarray(devices), ("core",))
    n_outs = len(out_names)
    sharded = jax.jit(
        shard_map(
            _body,
            mesh=mesh,
            in_specs=(PartitionSpec("core"),) * (n_params + n_outs),
            out_specs=(PartitionSpec("core"),) * n_outs,
            check_rep=False,
        ),
        donate_argnums=tuple(range(n_params, n_params + n_outs)),
        keep_unused=True,
    )
    sh = NamedSharding(mesh, PartitionSpec("core"))
    yi = out_names.index("y")

    def run(x_full, shared):
        concat = []
        for name in in_names:
            if name == "x":
                concat.append(x_full.reshape(B * S, DIM))
            else:
                concat.append(np.concatenate([shared[name]] * B, axis=0))
        dev_in = [jax.device_put(a, sh) for a in concat]
        zeros = [
            jax.device_put(np.zeros((B * z[0][0], *z[0][1:]), z[1]), sh)
            for z in zero_shapes
        ]
        outs = sharded(*dev_in, *zeros)
        y = np.asarray(outs[yi]).reshape(B, S, DIM)
        return y

    _runners[flags] = run
    return run


if __name__ == "__main__":
    rng = np.random.default_rng(0)
    inp = {
        "x": rng.standard_normal((B, S, DIM), dtype=np.float32),
        "router_w": (rng.standard_normal((DIM, E)) * 0.02).astype(np.float32),
        "router_b": np.zeros(E, np.float32),
        "expert_w": (rng.standard_normal((E, DIM, DIM)) * 0.02).astype(np.float32),
        "expert_b": np.zeros((E, DIM), np.float32),
        "out_w": (rng.standard_normal((DIM, DIM)) * 0.02).astype(np.float32),
        "out_b": np.zeros(DIM, np.float32),
        "norm_w": np.ones(DIM, np.float32),
    }
    y = kernel(**inp)
    print("kernel ran, y shape", y.shape, "finite:", np.isfinite(y).all())
